# revision 36
# baseline (speedup 1.0000x reference)
"""Trainium2 Bass kernel for nn_AttnConv2d (attention-conv + dynamic conv + BN).

Math (per sample b):
  a1 = conv3x3(x, w1); a2 = conv3x3(x, w2); a3 = conv3x3(x, w3)     (SAME pad)
  attn[h,w,i,o] = sum_{p,q} a1[i,3p+h,3q+w] * a2[o,3p+h,3q+w]
  kern[o,:,:,:] = softmax(attn[.,.,.,o] / sqrt(Ci*9))
  av = conv3x3(a3, kern[b])                                         (per-sample kernel)
  y  = feature_map_stack(av)   (pure spatial/channel permutation)
  out = cm * x + NORM_SCALE * (y - mean_y) * rsqrt(var_y + eps)     (batch stats)

Sharding: data-parallel over batch, 1 sample per core, 8 cores.  The only
cross-core exchange is an AllReduce of the per-channel BN partial sums,
split in two (strips 0-4 early so it overlaps pass B; strips 5-7 at the end).

Implementation notes:
  - everything runs bf16 on the PE (f32 PSUM accumulate).  bf16 weight loads
    (~119ns) hide under 384-col matmuls (~160ns), unlike fp32 loads (~227ns).
  - conv matmuls stream their rhs columns in subgrid-major order (pass A,
    for the attention gather) or parity-major order (pass B, for the
    feature_map_stack split), so every PSUM->SBUF copy has contiguous
    64/96-element inner runs on both sides.
  - attention needs positions on the partition axis: a1g/a2g strips are
    transposed by the DMA XBAR (dma_start_transpose, [128,512]->[128,4,128])
    instead of the PE; attn matmuls for strip s issue after the convs of
    strip s+1 so the PE never waits on the transpose DMA.
  - feature_map_stack is folded into a partition-strided SBUF->SBUF scatter
    DMA; av stays on-chip, pass C reads it directly.
  - pass C is fused: Act does sc*av+bb (per-partition scale/bias Identity),
    DVE adds the bf16 residual, output is written bf16 (host casts to f32).
"""

import os
import sys

for _p in ("/opt/trn_rl_repo", "/root/.axon_site/_ro/trn_rl_repo"):
    if os.path.isdir(_p) and _p not in sys.path:
        sys.path.insert(0, _p)
        break

import numpy as np

import concourse.bass as bass
import concourse.bacc as bacc
import concourse.tile as tile
from concourse import mybir

F32 = mybir.dt.float32
BF16 = mybir.dt.bfloat16

ATTN_K = 3
NH = 2
EPS = 1e-5
NORM_SCALE = 0.1816
CI = 128
CO = 128


def _rap(base, dims, off=0):
    """Raw AP on the same tensor as `base` (keeps base's partition dim)."""
    return bass.AP(tensor=base.tensor, offset=base.offset + off,
                   ap=[base.ap[0]] + [list(d) for d in dims])


def build_nc(H, W, R, n_cores, cm, level=5):
    """Build the per-core Bass kernel. R = strip rows (div by 6, even)."""
    assert H % R == 0 and R % 6 == 0 and W % 6 == 0
    NS = H // R                      # strips
    Wq = W // 3                      # attn subgrid cols
    P = (R // 3) * Wq                # attn positions per offset per strip
    S = H // 2                       # quadrant size of feature_map_stack
    NT = R // 2                      # psum tiles (2 rows) per strip
    Wh = W // 2
    PQ = NT * Wh                     # parity-split positions per strip
    N_TOT = float(n_cores * H * W)   # BN count per channel
    SCL = 1.0 / float(np.sqrt(CI * 9))
    AR1 = 5                          # strips in the first (overlapped) AllReduce
    WP = W + 2                       # padded row length
    assert P % 128 == 0
    NE = P // 128                    # 128-position chunks per offset per strip

    nc = bacc.Bacc("TRN2", target_bir_lowering=False, debug=False,
                   num_devices=n_cores)

    x_in = nc.dram_tensor("x", [128, H + 2, WP], BF16,
                          kind="ExternalInput").ap()   # host-padded (+1 ring)
    w1_in = nc.dram_tensor("w1t", [128, 9, 128], BF16, kind="ExternalInput").ap()
    w2_in = nc.dram_tensor("w2t", [128, 9, 128], BF16, kind="ExternalInput").ap()
    w3_in = nc.dram_tensor("w3t", [128, 9, 128], BF16, kind="ExternalInput").ap()
    id_in = nc.dram_tensor("ident", [128, 128], BF16, kind="ExternalInput").ap()
    gp_in = nc.dram_tensor("gsum", [128, 128], F32, kind="ExternalInput").ap()
    mk_in = nc.dram_tensor("mask4", [128, 4], F32, kind="ExternalInput").ap()
    out_d = nc.dram_tensor("out", [128, H, W], BF16, kind="ExternalOutput").ap()
    avp_d = nc.dram_tensor("avp", [128, H, W], BF16).ap()   # scratch, out layout

    with tile.TileContext(nc) as tc:
        consts = tc.alloc_tile_pool(name="consts", bufs=1)
        w1t = consts.tile([128, 9, 128], BF16, tag="w1t")
        w2t = consts.tile([128, 9, 128], BF16, tag="w2t")
        w3t = consts.tile([128, 9, 128], BF16, tag="w3t")
        ident = consts.tile([128, 128], BF16, tag="ident")
        gsum = consts.tile([128, 128], F32, tag="gsum")
        mask4 = consts.tile([128, 4], F32, tag="mask4")
        nc.sync.dma_start(out=w1t[:], in_=w1_in[:])
        nc.sync.dma_start(out=w2t[:], in_=w2_in[:])
        nc.sync.dma_start(out=w3t[:], in_=w3_in[:])
        nc.sync.dma_start(out=ident[:], in_=id_in[:])
        nc.sync.dma_start(out=gsum[:], in_=gp_in[:])
        nc.sync.dma_start(out=mask4[:], in_=mk_in[:])

        small = tc.alloc_tile_pool(name="small", bufs=1)
        stats_cols = small.tile([128, NS, 4, 2], F32, tag="stats_cols")
        sloc1 = small.tile([128, 8], F32, tag="sloc1")
        sloc2 = small.tile([128, 8], F32, tag="sloc2")
        sglob1 = small.tile([128, 8], F32, tag="sglob1")
        sglob2 = small.tile([128, 8], F32, tag="sglob2")
        sglob = small.tile([128, 8], F32, tag="sglob")
        scalars = small.tile([128, 16], F32, tag="scalars")
        msb = small.tile([128, 8], F32, tag="msb")
        sel = small.tile([128, 4], F32, tag="sel")

        kern_pool = tc.alloc_tile_pool(name="kern", bufs=1)
        kernT = [kern_pool.tile([128, 128], BF16, tag=f"kT{k}", name=f"kT{k}")
                 for k in range(9)]

        a3_pool = tc.alloc_tile_pool(name="a3p", bufs=1)
        a3p = a3_pool.tile([128, H + 2, WP], BF16, tag="a3p")
        # zero the pad border of a3p once
        nc.vector.memset(_rap(a3p[:], [[1, WP]]), 0.0)                      # row 0
        nc.vector.memset(_rap(a3p[:], [[1, WP]], (H + 1) * WP), 0.0)        # row H+1
        nc.vector.memset(_rap(a3p[:], [[WP, H + 2]]), 0.0)                  # col 0
        nc.vector.memset(_rap(a3p[:], [[WP, H + 2]], W + 1), 0.0)           # col W+1

        attn_psp = tc.alloc_tile_pool(name="attn_ps", bufs=1, space="PSUM")
        attn_ps = attn_psp.tile([128, 9 * 128], F32, tag="attn")

        # ---------------- pass A: static convs + attention accumulation ------
        pa_x = tc.alloc_tile_pool(name="pa_x", bufs=2)
        pa_g = tc.alloc_tile_pool(name="pa_g", bufs=2)
        pa_t = tc.alloc_tile_pool(name="pa_t", bufs=20)
        pa_cps = tc.alloc_tile_pool(name="pa_cps", bufs=5, space="PSUM")

        aT_tiles = {}

        def emit_attn_mms(s):
            a1T, a2T = aT_tiles.pop(s)
            for hw in range(9):
                for e in range(NE):
                    nc.tensor.matmul(
                        attn_ps[:, hw * 128:(hw + 1) * 128],
                        a2T[hw][:, e, :], a1T[hw][:, e, :],
                        start=(s == 0 and e == 0 and hw in (0, 4, 8)),
                        stop=(s == NS - 1 and e == NE - 1 and hw in (3, 7, 8)),
                        skip_group_check=True)

        xs_tiles = {}

        def load_xs(s):
            t = pa_x.tile([128, R + 2, WP], BF16, tag="xs")
            nc.sync.dma_start(out=t[:], in_=x_in[:, s * R:s * R + R + 2, :])
            xs_tiles[s] = t

        load_xs(0)
        for s in range(NS):
            y0 = s * R
            if s + 1 < NS:
                load_xs(s + 1)   # issue before this strip's dependent DMAs
            xs = xs_tiles.pop(s)

            a1g = pa_g.tile([128, 9, P], BF16, tag="a1g")
            a2g = pa_g.tile([128, 9, P], BF16, tag="a2g")
            for t in range(NT):
                for wt, gdst in ((w1t, a1g), (w2t, a2g), (w3t, None)):
                    cps = pa_cps.tile([128, 2 * W], F32, tag="cps")
                    for k in range(9):
                        dy, dx = divmod(k, 3)
                        rhs = xs[:, 2 * t + dy:2 * t + dy + 2, dx:dx + W]
                        nc.tensor.matmul(cps[:, :], wt[:, k, :], rhs,
                                         start=(k == 0), stop=(k == 8))
                    if gdst is not None:
                        # scatter rows (2t, 2t+1) into subgrid-major layout
                        ya, yb = 2 * t, 2 * t + 1
                        ha, ra = ya % 3, ya // 3
                        hb, rb = yb % 3, yb // 3
                        offa = (3 * ha) * P + ra * Wq
                        sd = (3 * hb) * P + rb * Wq - offa
                        src = _rap(cps[:], [[W, 2], [1, 3], [3, Wq]])
                        dst = _rap(gdst[:], [[sd, 2], [P, 3], [1, Wq]], offa)
                        if gdst is a1g:
                            nc.vector.tensor_copy(dst, src)
                        else:
                            nc.scalar.copy(out=dst, in_=src)
                    else:
                        nc.scalar.copy(
                            out=a3p[:, 1 + y0 + 2 * t:1 + y0 + 2 * t + 2, 1:1 + W],
                            in_=_rap(cps[:], [[W, 2], [1, W]]))
            # XBAR transposes for this strip (run during next strip's convs)
            a1T = []
            a2T = []
            for hw in range(9):
                t1 = pa_t.tile([128, NE, 128], BF16, tag="a1T")
                nc.sync.dma_start_transpose(out=t1[:], in_=a1g[:, hw, :])
                a1T.append(t1)
                t2 = pa_t.tile([128, NE, 128], BF16, tag="a2T")
                nc.sync.dma_start_transpose(out=t2[:], in_=a2g[:, hw, :])
                a2T.append(t2)
            aT_tiles[s] = (a1T, a2T)
            if s >= 1:
                emit_attn_mms(s - 1)
        emit_attn_mms(NS - 1)

        pa_cps.release()
        pa_t.release(); pa_g.release(); pa_x.release()

        # ---------------- softmax + kern transposes -------------------------
        if level >= 2:
            sm_pool = tc.alloc_tile_pool(name="smx", bufs=1)
            ssum = scalars[:, 2:3]
            rsum = scalars[:, 3:4]
            # logits*SCL are bounded (|attn| < ~10 sigma -> exp < e^19), so
            # the max-subtraction is unnecessary; exp reads PSUM directly.
            esb = sm_pool.tile([128, 9 * 128], F32, tag="esb")
            nc.scalar.activation(esb[:], attn_ps[:],
                                 mybir.ActivationFunctionType.Exp,
                                 scale=SCL)
            attn_psp.release()
            nc.vector.reduce_sum(ssum, esb[:], axis=mybir.AxisListType.X)
            nc.vector.reciprocal(rsum, ssum)
            sm_bf = sm_pool.tile([128, 9 * 128], BF16, tag="sm_bf")
            nc.vector.tensor_scalar_mul(sm_bf[:], esb[:], rsum)
            k_tps = tc.alloc_tile_pool(name="k_tps", bufs=2, space="PSUM")
            for hw in range(9):
                tp = k_tps.tile([128, 128], BF16, tag="ktp")
                nc.tensor.transpose(tp[:], sm_bf[:, hw * 128:(hw + 1) * 128],
                                    ident[:])
                nc.vector.tensor_copy(kernT[hw][:], tp[:])
            k_tps.release()
            sm_pool.release()
        else:
            attn_psp.release()

        # ---------------- pass B: dynamic conv + stats + permuted store -----
        # pass C strips in avp-availability order: out strip t (and t+4) is
        # fully written once pass B strip 2t+1 is scattered.
        PC_ORDER = [t for pair in zip(range(NS // 2), range(NS // 2, NS))
                    for t in pair]
        if level >= 3:
            pb_av = tc.alloc_tile_pool(name="pb_av", bufs=4)
            pb_sq = tc.alloc_tile_pool(name="pb_sq", bufs=1)
            pb_cps = tc.alloc_tile_pool(name="pb_cps", bufs=6, space="PSUM")
            cc_pool = tc.alloc_tile_pool(name="ccd", bufs=1, space="DRAM")
            cc_in1 = cc_pool.tile([128, 8], F32, tag="cc_in1")
            cc_out1 = cc_pool.tile([128, 8], F32, tag="cc_out1")
            cc_inb = cc_pool.tile([128, 1], F32, tag="cc_inb")
            cc_outb = cc_pool.tile([128, 1], F32, tag="cc_outb")
            nc.gpsimd.dma_start(out=cc_inb[:], in_=kernT[0][:, 0:1])
            nc.gpsimd.collective_compute(
                "AllReduce", mybir.AluOpType.add,
                replica_groups=[list(range(n_cores))],
                ins=[cc_inb.opt()], outs=[cc_outb.opt()])
            nc.gpsimd.dma_start(out=sglob1[:, 0:1], in_=cc_outb[:])
            for s in range(NS):
                y0 = s * R
                # av parity-split: av_sp[c, 2i+j, t, q] = av[c, 2t+i, 2q+j]
                av_sp = pb_av.tile([128, 4, NT, Wh], BF16, tag="av")
                for t in range(NT):
                    cps = pb_cps.tile([128, 2 * W], F32, tag="cps2")
                    for k in range(9):
                        dy, dx = divmod(k, 3)
                        # parity-major columns: (row i, phase j, q)
                        rhs = _rap(a3p[:], [[WP, 2], [1, 2], [2, Wh]],
                                   (y0 + 2 * t + dy) * WP + dx)
                        nc.tensor.matmul(cps[:, :], kernT[k][:], rhs,
                                         start=(k == 0), stop=(k == 8))
                    src = _rap(cps[:], [[W, 2], [Wh, 2], [1, Wh]])
                    dst = _rap(av_sp[:], [[2 * PQ, 2], [PQ, 2], [1, Wh]],
                               t * Wh)
                    if t % 2 == 0:
                        nc.vector.tensor_copy(dst, src)
                    else:
                        nc.scalar.copy(out=dst, in_=src)
                sq = pb_sq.tile([128, PQ], BF16, tag="sq")
                for pi in range(4):
                    psrc = _rap(av_sp[:], [[1, PQ]], pi * PQ)
                    nc.vector.reduce_sum(stats_cols[:, s, pi, 0:1], psrc,
                                         axis=mybir.AxisListType.X)
                    nc.scalar.activation(
                        out=sq[:], in_=psrc,
                        func=mybir.ActivationFunctionType.Square,
                        accum_out=stats_cols[:, s, pi, 1:2])
                # feature_map_stack fold: av[32*c2+c1, 2t+i, 2q+j]
                #   -> avp[4*c1 + 2i+j, S*(c2>>1) + (s*NT+t), S*(c2&1) + q]
                for pi, (i, j) in enumerate(((0, 0), (0, 1), (1, 0), (1, 1))):
                    for c2 in range(4):
                        qsrc = _rap(av_sp[32 * c2:32 * (c2 + 1)],
                                    [[Wh, NT], [1, Wh]], pi * PQ)
                        dsto = (pi * H * W + (c2 >> 1) * S * W
                                + (c2 & 1) * S + (s * NT) * W)
                        dst = bass.AP(tensor=avp_d.tensor,
                                      offset=avp_d.offset + dsto,
                                      ap=[[4 * H * W, 32], [W, NT], [1, Wh]])
                        nc.sync.dma_start(out=dst, in_=qsrc)
            pb_cps.release()
            pb_sq.release(); pb_av.release()

        # ---------------- AllReduce + BN coefficients -----------------------
        if level >= 4:
            nc.vector.reduce_sum(
                _rap(sglob[:], [[2, 4], [1, 2]]),
                _rap(stats_cols[:], [[2, 4], [1, 2], [8, NS]]),
                axis=mybir.AxisListType.X)
            nc.gpsimd.dma_start(out=cc_in1[:], in_=sglob[:])
            nc.gpsimd.collective_compute(
                "AllReduce", mybir.AluOpType.add,
                replica_groups=[list(range(n_cores))],
                ins=[cc_in1.opt()], outs=[cc_out1.opt()])
            nc.gpsimd.dma_start(out=sglob[:], in_=cc_out1[:])

            # a3p/kern are dead; free them so pass C gets deep buffer pools,
            # and issue every pass-C input DMA now — they run under the AR.
            a3_pool.release()
            kern_pool.release()
            if level >= 5:
                pc_a = tc.alloc_tile_pool(name="pc_a", bufs=8)
                pc_x = tc.alloc_tile_pool(name="pc_x", bufs=6)
                pca_tiles = {}
                pcx_tiles = {}

                def load_pcx(t):
                    x_s = pc_x.tile([128, R, W], BF16, tag="x_s")
                    nc.sync.dma_start(
                        out=x_s[:],
                        in_=x_in[:, 1 + t * R:1 + t * R + R, 1:1 + W])
                    pcx_tiles[t] = x_s

                for t in PC_ORDER[:6]:
                    load_pcx(t)              # dep-free, fire immediately
                for t in PC_ORDER:
                    av_s = pc_a.tile([128, R, W], BF16, tag="av_s")
                    nc.sync.dma_start(out=av_s[:],
                                      in_=avp_d[:, t * R:(t + 1) * R, :])
                    pca_tiles[t] = av_s
                for t in PC_ORDER[6:]:
                    load_pcx(t)

            # ------------ BN coefficients (per out-channel) -----------------
            bn_ps = tc.alloc_tile_pool(name="bn_ps", bufs=1, space="PSUM")
            gps = bn_ps.tile([128, 8], F32, tag="gps")
            nc.tensor.matmul(gps[:], gsum[:], sglob[:], start=True, stop=True)
            nc.vector.tensor_copy(msb[:], gps[:])
            bn_ps.release()
            mean = scalars[:, 4:5]
            e2 = scalars[:, 5:6]
            msq = scalars[:, 6:7]
            var = scalars[:, 7:8]
            sd = scalars[:, 8:9]
            rstd = scalars[:, 9:10]
            sc = scalars[:, 10:11]
            bb0 = scalars[:, 11:12]
            bb = scalars[:, 12:13]
            nc.vector.tensor_mul(sel[:], _rap(msb[:], [[2, 4]]), mask4[:])
            nc.vector.reduce_sum(mean, sel[:], axis=mybir.AxisListType.X)
            nc.vector.tensor_scalar_mul(mean, mean, 1.0 / N_TOT)
            nc.vector.tensor_mul(sel[:], _rap(msb[:], [[2, 4]], 1), mask4[:])
            nc.vector.reduce_sum(e2, sel[:], axis=mybir.AxisListType.X)
            nc.vector.tensor_scalar_mul(e2, e2, 1.0 / N_TOT)
            nc.vector.tensor_mul(msq, mean, mean)
            nc.vector.tensor_tensor(out=var, in0=e2, in1=msq,
                                    op=mybir.AluOpType.subtract)
            eps_ap = scalars[:, 13:14]
            nc.vector.memset(eps_ap, EPS)
            nc.scalar.activation(sd, var, mybir.ActivationFunctionType.Sqrt,
                                 bias=eps_ap)
            nc.vector.reciprocal(rstd, sd)
            nc.vector.tensor_scalar_mul(sc, rstd, NORM_SCALE)
            nc.vector.tensor_mul(bb0, mean, sc)
            nc.vector.tensor_scalar_mul(bb, bb0, -1.0)

        # ---------------- pass C: out = cm*x + sc*avp + bb ------------------
        if level >= 5:
            pc_t = tc.alloc_tile_pool(name="pc_t", bufs=2)
            pc_o = tc.alloc_tile_pool(name="pc_o", bufs=4)
            for idx, t in enumerate(PC_ORDER):
                y0 = t * R
                x_s = pcx_tiles.pop(t)
                if cm != 1.0:
                    xc = pc_t.tile([128, R, W], BF16, tag="xc")
                    nc.scalar.mul(xc[:], x_s[:], float(cm))
                    x_s = xc
                av_s = pca_tiles.pop(t)
                # t = sc*av + bb (Act and DVE alternate strips), out = t + x.
                t_s = pc_t.tile([128, R, W], BF16, tag="t_s")
                if idx % 2 == 0:
                    nc.scalar.activation(t_s[:], av_s[:],
                                         mybir.ActivationFunctionType.Identity,
                                         bias=bb, scale=sc)
                else:
                    nc.vector.tensor_scalar(out=t_s[:], in0=av_s[:],
                                            scalar1=sc, scalar2=bb,
                                            op0=mybir.AluOpType.mult,
                                            op1=mybir.AluOpType.add)
                o_s = pc_o.tile([128, R, W], BF16, tag="o_s")
                nc.vector.tensor_add(o_s[:], t_s[:], x_s[:])
                nc.sync.dma_start(out=out_d[:, y0:y0 + R, :], in_=o_s[:])
            pc_o.release(); pc_t.release()

        if level >= 3:
            cc_pool.release()
        if level >= 5:
            pc_x.release(); pc_a.release()
        small.release()
        consts.release()

    nc.compile()
    return nc


def _prep_wt(w, permute_out=False):
    """[Co,Ci,3,3] -> lhsT layout [Ci, 9, Co] (optionally out-chan permuted)."""
    import ml_dtypes
    wt = np.ascontiguousarray(w.transpose(1, 2, 3, 0).reshape(128, 9, 128))
    if permute_out:
        p = np.arange(128)
        co_of_p = 4 * (p % 32) + p // 32     # partition p holds channel co_of_p
        wt = np.ascontiguousarray(wt[:, :, co_of_p])
    return wt.astype(ml_dtypes.bfloat16)


def make_const_inputs(w1, w2, w3):
    import ml_dtypes
    p = np.arange(128)
    # gsum[p_src, C']: sum av partitions with p_src%32 == C'//4
    gsum = (p[:, None] % 32 == p[None, :] // 4).astype(np.float32)
    mask4 = (p[:, None] % 4 == np.arange(4)[None, :]).astype(np.float32)
    return {
        "ident": np.eye(128, dtype=np.float32).astype(ml_dtypes.bfloat16),
        "w1t": _prep_wt(np.asarray(w1, np.float32)),
        "w2t": _prep_wt(np.asarray(w2, np.float32), permute_out=True),
        "w3t": _prep_wt(np.asarray(w3, np.float32)),
        "gsum": gsum,
        "mask4": mask4,
    }


def pad_x(x_sample):
    import ml_dtypes
    return np.pad(x_sample, ((0, 0), (1, 1), (1, 1))).astype(ml_dtypes.bfloat16)


_CACHE = {}


def kernel(x, w1, w2, w3, conv_momentum):
    from concourse.bass_utils import run_bass_kernel_spmd

    x = np.asarray(x, np.float32)
    B, Ci, H, W = x.shape
    cm = float(np.asarray(conv_momentum))
    key = (H, W, B, cm)
    if key not in _CACHE:
        _CACHE[key] = build_nc(H, W, 24, B, cm)
    nc = _CACHE[key]
    consts = make_const_inputs(w1, w2, w3)
    in_maps = [dict(consts, x=pad_x(x[b])) for b in range(B)]
    res = run_bass_kernel_spmd(nc, in_maps, list(range(B)))
    out = np.stack(
        [np.asarray(res.results[b]["out"]).reshape(128, H, W) for b in range(B)],
        axis=0)
    return out.astype(np.float32)


# revision 37
# speedup vs baseline: 1.0103x; 1.0103x over previous
"""Trainium2 Bass kernel for nn_AttnConv2d (attention-conv + dynamic conv + BN).

Math (per sample b):
  a1 = conv3x3(x, w1); a2 = conv3x3(x, w2); a3 = conv3x3(x, w3)     (SAME pad)
  attn[h,w,i,o] = sum_{p,q} a1[i,3p+h,3q+w] * a2[o,3p+h,3q+w]
  kern[o,:,:,:] = softmax(attn[.,.,.,o] / sqrt(Ci*9))
  av = conv3x3(a3, kern[b])                                         (per-sample kernel)
  y  = feature_map_stack(av)   (pure spatial/channel permutation)
  out = cm * x + NORM_SCALE * (y - mean_y) * rsqrt(var_y + eps)     (batch stats)

Sharding: data-parallel over batch, 1 sample per core, 8 cores.  The only
cross-core exchange is an AllReduce of the per-channel BN partial sums,
split in two (strips 0-4 early so it overlaps pass B; strips 5-7 at the end).

Implementation notes:
  - everything runs bf16 on the PE (f32 PSUM accumulate).  bf16 weight loads
    (~119ns) hide under 384-col matmuls (~160ns), unlike fp32 loads (~227ns).
  - conv matmuls stream their rhs columns in subgrid-major order (pass A,
    for the attention gather) or parity-major order (pass B, for the
    feature_map_stack split), so every PSUM->SBUF copy has contiguous
    64/96-element inner runs on both sides.
  - attention needs positions on the partition axis: a1g/a2g strips are
    transposed by the DMA XBAR (dma_start_transpose, [128,512]->[128,4,128])
    instead of the PE; attn matmuls for strip s issue after the convs of
    strip s+1 so the PE never waits on the transpose DMA.
  - feature_map_stack is folded into a partition-strided SBUF->SBUF scatter
    DMA; av stays on-chip, pass C reads it directly.
  - pass C is fused: Act does sc*av+bb (per-partition scale/bias Identity),
    DVE adds the bf16 residual, output is written bf16 (host casts to f32).
"""

import os
import sys

for _p in ("/opt/trn_rl_repo", "/root/.axon_site/_ro/trn_rl_repo"):
    if os.path.isdir(_p) and _p not in sys.path:
        sys.path.insert(0, _p)
        break

import numpy as np

import concourse.bass as bass
import concourse.bacc as bacc
import concourse.tile as tile
from concourse import mybir

F32 = mybir.dt.float32
BF16 = mybir.dt.bfloat16

ATTN_K = 3
NH = 2
EPS = 1e-5
NORM_SCALE = 0.1816
CI = 128
CO = 128


def _rap(base, dims, off=0):
    """Raw AP on the same tensor as `base` (keeps base's partition dim)."""
    return bass.AP(tensor=base.tensor, offset=base.offset + off,
                   ap=[base.ap[0]] + [list(d) for d in dims])


def build_nc(H, W, R, n_cores, cm, level=5):
    """Build the per-core Bass kernel. R = strip rows (div by 6, even)."""
    assert H % R == 0 and R % 6 == 0 and W % 6 == 0
    NS = H // R                      # strips
    Wq = W // 3                      # attn subgrid cols
    P = (R // 3) * Wq                # attn positions per offset per strip
    S = H // 2                       # quadrant size of feature_map_stack
    NT = R // 2                      # psum tiles (2 rows) per strip
    Wh = W // 2
    PQ = NT * Wh                     # parity-split positions per strip
    N_TOT = float(n_cores * H * W)   # BN count per channel
    SCL = 1.0 / float(np.sqrt(CI * 9))
    AR1 = 5                          # strips in the first (overlapped) AllReduce
    WP = W + 2                       # padded row length
    assert P % 128 == 0
    NE = P // 128                    # 128-position chunks per offset per strip

    nc = bacc.Bacc("TRN2", target_bir_lowering=False, debug=False,
                   num_devices=n_cores)

    x_in = nc.dram_tensor("x", [128, H + 2, WP], BF16,
                          kind="ExternalInput").ap()   # host-padded (+1 ring)
    w1_in = nc.dram_tensor("w1t", [128, 9, 128], BF16, kind="ExternalInput").ap()
    w2_in = nc.dram_tensor("w2t", [128, 9, 128], BF16, kind="ExternalInput").ap()
    w3_in = nc.dram_tensor("w3t", [128, 9, 128], BF16, kind="ExternalInput").ap()
    id_in = nc.dram_tensor("ident", [128, 128], BF16, kind="ExternalInput").ap()
    gp_in = nc.dram_tensor("gsum", [128, 128], F32, kind="ExternalInput").ap()
    mk_in = nc.dram_tensor("mask4", [128, 4], F32, kind="ExternalInput").ap()
    out_d = nc.dram_tensor("out", [128, H, W], BF16, kind="ExternalOutput").ap()
    avp_d = nc.dram_tensor("avp", [128, H, W], BF16).ap()   # scratch, out layout

    with tile.TileContext(nc) as tc:
        consts = tc.alloc_tile_pool(name="consts", bufs=1)
        w1t = consts.tile([128, 9, 128], BF16, tag="w1t")
        w2t = consts.tile([128, 9, 128], BF16, tag="w2t")
        w3t = consts.tile([128, 9, 128], BF16, tag="w3t")
        ident = consts.tile([128, 128], BF16, tag="ident")
        gsum = consts.tile([128, 128], F32, tag="gsum")
        mask4 = consts.tile([128, 4], F32, tag="mask4")
        nc.sync.dma_start(out=w1t[:], in_=w1_in[:])
        nc.sync.dma_start(out=w2t[:], in_=w2_in[:])
        nc.sync.dma_start(out=w3t[:], in_=w3_in[:])
        nc.sync.dma_start(out=ident[:], in_=id_in[:])
        nc.sync.dma_start(out=gsum[:], in_=gp_in[:])
        nc.sync.dma_start(out=mask4[:], in_=mk_in[:])

        small = tc.alloc_tile_pool(name="small", bufs=1)
        stats_cols = small.tile([128, NS, 4, 2], F32, tag="stats_cols")
        sloc1 = small.tile([128, 8], F32, tag="sloc1")
        sloc2 = small.tile([128, 8], F32, tag="sloc2")
        sglob1 = small.tile([128, 8], F32, tag="sglob1")
        sglob2 = small.tile([128, 8], F32, tag="sglob2")
        sglob = small.tile([128, 8], F32, tag="sglob")
        scalars = small.tile([128, 16], F32, tag="scalars")
        msb = small.tile([128, 8], F32, tag="msb")
        sel = small.tile([128, 4], F32, tag="sel")

        kern_pool = tc.alloc_tile_pool(name="kern", bufs=1)
        kernT = [kern_pool.tile([128, 128], BF16, tag=f"kT{k}", name=f"kT{k}")
                 for k in range(9)]

        a3_pool = tc.alloc_tile_pool(name="a3p", bufs=1)
        a3p = a3_pool.tile([128, H + 2, WP], BF16, tag="a3p")
        # zero the pad border of a3p once
        nc.vector.memset(_rap(a3p[:], [[1, WP]]), 0.0)                      # row 0
        nc.vector.memset(_rap(a3p[:], [[1, WP]], (H + 1) * WP), 0.0)        # row H+1
        nc.vector.memset(_rap(a3p[:], [[WP, H + 2]]), 0.0)                  # col 0
        nc.vector.memset(_rap(a3p[:], [[WP, H + 2]], W + 1), 0.0)           # col W+1

        attn_psp = tc.alloc_tile_pool(name="attn_ps", bufs=1, space="PSUM")
        attn_ps = attn_psp.tile([128, 9 * 128], F32, tag="attn")

        # ---------------- pass A: static convs + attention accumulation ------
        pa_x = tc.alloc_tile_pool(name="pa_x", bufs=2)
        pa_g = tc.alloc_tile_pool(name="pa_g", bufs=2)
        pa_t = tc.alloc_tile_pool(name="pa_t", bufs=20)
        pa_cps = tc.alloc_tile_pool(name="pa_cps", bufs=5, space="PSUM")

        aT_tiles = {}

        def emit_attn_mms(s):
            a1T, a2T = aT_tiles.pop(s)
            for hw in range(9):
                for e in range(NE):
                    nc.tensor.matmul(
                        attn_ps[:, hw * 128:(hw + 1) * 128],
                        a2T[hw][:, e, :], a1T[hw][:, e, :],
                        start=(s == 0 and e == 0 and hw in (0, 4, 8)),
                        stop=(s == NS - 1 and e == NE - 1 and hw in (3, 7, 8)),
                        skip_group_check=True)

        xs_tiles = {}

        def load_xs(s):
            t = pa_x.tile([128, R + 2, WP], BF16, tag="xs")
            nc.sync.dma_start(out=t[:], in_=x_in[:, s * R:s * R + R + 2, :])
            xs_tiles[s] = t

        load_xs(0)
        for s in range(NS):
            y0 = s * R
            if s + 1 < NS:
                load_xs(s + 1)   # issue before this strip's dependent DMAs
            xs = xs_tiles.pop(s)

            a1g = pa_g.tile([128, 9, P], BF16, tag="a1g")
            a2g = pa_g.tile([128, 9, P], BF16, tag="a2g")
            for t in range(NT):
                for wt, gdst in ((w1t, a1g), (w2t, a2g), (w3t, None)):
                    cps = pa_cps.tile([128, 2 * W], F32, tag="cps")
                    for k in range(9):
                        dy, dx = divmod(k, 3)
                        rhs = xs[:, 2 * t + dy:2 * t + dy + 2, dx:dx + W]
                        nc.tensor.matmul(cps[:, :], wt[:, k, :], rhs,
                                         start=(k == 0), stop=(k == 8))
                    if gdst is not None:
                        # scatter rows (2t, 2t+1) into subgrid-major layout
                        ya, yb = 2 * t, 2 * t + 1
                        ha, ra = ya % 3, ya // 3
                        hb, rb = yb % 3, yb // 3
                        offa = (3 * ha) * P + ra * Wq
                        sd = (3 * hb) * P + rb * Wq - offa
                        src = _rap(cps[:], [[W, 2], [1, 3], [3, Wq]])
                        dst = _rap(gdst[:], [[sd, 2], [P, 3], [1, Wq]], offa)
                        if gdst is a1g:
                            nc.vector.tensor_copy(dst, src)
                        else:
                            nc.scalar.copy(out=dst, in_=src)
                    else:
                        nc.scalar.copy(
                            out=a3p[:, 1 + y0 + 2 * t:1 + y0 + 2 * t + 2, 1:1 + W],
                            in_=_rap(cps[:], [[W, 2], [1, W]]))
            # XBAR transposes for this strip (run during next strip's convs)
            a1T = []
            a2T = []
            for hw in range(9):
                t1 = pa_t.tile([128, NE, 128], BF16, tag="a1T")
                nc.sync.dma_start_transpose(out=t1[:], in_=a1g[:, hw, :])
                a1T.append(t1)
                t2 = pa_t.tile([128, NE, 128], BF16, tag="a2T")
                nc.sync.dma_start_transpose(out=t2[:], in_=a2g[:, hw, :])
                a2T.append(t2)
            aT_tiles[s] = (a1T, a2T)
            if s >= 1:
                emit_attn_mms(s - 1)
        emit_attn_mms(NS - 1)

        pa_cps.release()
        pa_t.release(); pa_g.release(); pa_x.release()

        # ---------------- softmax + kern transposes -------------------------
        if level >= 2:
            sm_pool = tc.alloc_tile_pool(name="smx", bufs=1)
            ssum = scalars[:, 2:3]
            rsum = scalars[:, 3:4]
            # logits*SCL are bounded (|attn| < ~10 sigma -> exp < e^19), so
            # the max-subtraction is unnecessary; exp reads PSUM directly.
            esb = sm_pool.tile([128, 9 * 128], F32, tag="esb")
            nc.scalar.activation(esb[:], attn_ps[:],
                                 mybir.ActivationFunctionType.Exp,
                                 scale=SCL)
            attn_psp.release()
            nc.vector.reduce_sum(ssum, esb[:], axis=mybir.AxisListType.X)
            nc.vector.reciprocal(rsum, ssum)
            sm_bf = sm_pool.tile([128, 9 * 128], BF16, tag="sm_bf")
            nc.vector.tensor_scalar_mul(sm_bf[:], esb[:], rsum)
            k_tps = tc.alloc_tile_pool(name="k_tps", bufs=2, space="PSUM")
            for hw in range(9):
                tp = k_tps.tile([128, 128], BF16, tag="ktp")
                nc.tensor.transpose(tp[:], sm_bf[:, hw * 128:(hw + 1) * 128],
                                    ident[:])
                nc.vector.tensor_copy(kernT[hw][:], tp[:])
            k_tps.release()
            sm_pool.release()
        else:
            attn_psp.release()

        # ---------------- pass B: dynamic conv + stats + permuted store -----
        # pass C strips in avp-availability order: out strip t (and t+4) is
        # fully written once pass B strip 2t+1 is scattered.
        PC_ORDER = [t for pair in zip(range(NS // 2), range(NS // 2, NS))
                    for t in pair]
        if level >= 3:
            pb_av = tc.alloc_tile_pool(name="pb_av", bufs=4)
            pb_sq = tc.alloc_tile_pool(name="pb_sq", bufs=1)
            pb_cps = tc.alloc_tile_pool(name="pb_cps", bufs=7, space="PSUM")
            cc_pool = tc.alloc_tile_pool(name="ccd", bufs=1, space="DRAM")
            cc_in1 = cc_pool.tile([128, 8], F32, tag="cc_in1")
            cc_out1 = cc_pool.tile([128, 8], F32, tag="cc_out1")
            cc_inb = cc_pool.tile([128, 1], F32, tag="cc_inb")
            cc_outb = cc_pool.tile([128, 1], F32, tag="cc_outb")
            nc.gpsimd.dma_start(out=cc_inb[:], in_=kernT[0][:, 0:1])
            nc.gpsimd.collective_compute(
                "AllReduce", mybir.AluOpType.add,
                replica_groups=[list(range(n_cores))],
                ins=[cc_inb.opt()], outs=[cc_outb.opt()])
            nc.gpsimd.dma_start(out=sglob1[:, 0:1], in_=cc_outb[:])
            for s in range(NS):
                y0 = s * R
                # av parity-split: av_sp[c, 2i+j, t, q] = av[c, 2t+i, 2q+j]
                av_sp = pb_av.tile([128, 4, NT, Wh], BF16, tag="av")
                for t in range(NT):
                    cps = pb_cps.tile([128, 2 * W], F32, tag="cps2")
                    for k in range(9):
                        dy, dx = divmod(k, 3)
                        # parity-major columns: (row i, phase j, q)
                        rhs = _rap(a3p[:], [[WP, 2], [1, 2], [2, Wh]],
                                   (y0 + 2 * t + dy) * WP + dx)
                        nc.tensor.matmul(cps[:, :], kernT[k][:], rhs,
                                         start=(k == 0), stop=(k == 8))
                    src = _rap(cps[:], [[W, 2], [Wh, 2], [1, Wh]])
                    dst = _rap(av_sp[:], [[2 * PQ, 2], [PQ, 2], [1, Wh]],
                               t * Wh)
                    if t % 2 == 0:
                        nc.vector.tensor_copy(dst, src)
                    else:
                        nc.scalar.copy(out=dst, in_=src)
                sq = pb_sq.tile([128, PQ], BF16, tag="sq")
                for pi in range(4):
                    psrc = _rap(av_sp[:], [[1, PQ]], pi * PQ)
                    nc.vector.reduce_sum(stats_cols[:, s, pi, 0:1], psrc,
                                         axis=mybir.AxisListType.X)
                    nc.scalar.activation(
                        out=sq[:], in_=psrc,
                        func=mybir.ActivationFunctionType.Square,
                        accum_out=stats_cols[:, s, pi, 1:2])
                # feature_map_stack fold: av[32*c2+c1, 2t+i, 2q+j]
                #   -> avp[4*c1 + 2i+j, S*(c2>>1) + (s*NT+t), S*(c2&1) + q]
                for pi, (i, j) in enumerate(((0, 0), (0, 1), (1, 0), (1, 1))):
                    for c2 in range(4):
                        qsrc = _rap(av_sp[32 * c2:32 * (c2 + 1)],
                                    [[Wh, NT], [1, Wh]], pi * PQ)
                        dsto = (pi * H * W + (c2 >> 1) * S * W
                                + (c2 & 1) * S + (s * NT) * W)
                        dst = bass.AP(tensor=avp_d.tensor,
                                      offset=avp_d.offset + dsto,
                                      ap=[[4 * H * W, 32], [W, NT], [1, Wh]])
                        nc.sync.dma_start(out=dst, in_=qsrc)
            pb_cps.release()
            pb_sq.release(); pb_av.release()

        # ---------------- AllReduce + BN coefficients -----------------------
        if level >= 4:
            nc.vector.reduce_sum(
                _rap(sglob[:], [[2, 4], [1, 2]]),
                _rap(stats_cols[:], [[2, 4], [1, 2], [8, NS]]),
                axis=mybir.AxisListType.X)
            nc.gpsimd.dma_start(out=cc_in1[:], in_=sglob[:])
            nc.gpsimd.collective_compute(
                "AllReduce", mybir.AluOpType.add,
                replica_groups=[list(range(n_cores))],
                ins=[cc_in1.opt()], outs=[cc_out1.opt()])
            nc.gpsimd.dma_start(out=sglob[:], in_=cc_out1[:])

            # a3p/kern are dead; free them so pass C gets deep buffer pools,
            # and issue every pass-C input DMA now — they run under the AR.
            a3_pool.release()
            kern_pool.release()
            if level >= 5:
                pc_a = tc.alloc_tile_pool(name="pc_a", bufs=8)
                pc_x = tc.alloc_tile_pool(name="pc_x", bufs=6)
                pca_tiles = {}
                pcx_tiles = {}

                def load_pcx(t):
                    x_s = pc_x.tile([128, R, W], BF16, tag="x_s")
                    nc.sync.dma_start(
                        out=x_s[:],
                        in_=x_in[:, 1 + t * R:1 + t * R + R, 1:1 + W])
                    pcx_tiles[t] = x_s

                for t in PC_ORDER[:6]:
                    load_pcx(t)              # dep-free, fire immediately
                for t in PC_ORDER:
                    av_s = pc_a.tile([128, R, W], BF16, tag="av_s")
                    nc.sync.dma_start(out=av_s[:],
                                      in_=avp_d[:, t * R:(t + 1) * R, :])
                    pca_tiles[t] = av_s
                for t in PC_ORDER[6:]:
                    load_pcx(t)

            # ------------ BN coefficients (per out-channel) -----------------
            bn_ps = tc.alloc_tile_pool(name="bn_ps", bufs=1, space="PSUM")
            gps = bn_ps.tile([128, 8], F32, tag="gps")
            nc.tensor.matmul(gps[:], gsum[:], sglob[:], start=True, stop=True)
            nc.vector.tensor_copy(msb[:], gps[:])
            bn_ps.release()
            mean = scalars[:, 4:5]
            e2 = scalars[:, 5:6]
            msq = scalars[:, 6:7]
            var = scalars[:, 7:8]
            sd = scalars[:, 8:9]
            rstd = scalars[:, 9:10]
            sc = scalars[:, 10:11]
            bb0 = scalars[:, 11:12]
            bb = scalars[:, 12:13]
            nc.vector.tensor_mul(sel[:], _rap(msb[:], [[2, 4]]), mask4[:])
            nc.vector.reduce_sum(mean, sel[:], axis=mybir.AxisListType.X)
            nc.vector.tensor_scalar_mul(mean, mean, 1.0 / N_TOT)
            nc.vector.tensor_mul(sel[:], _rap(msb[:], [[2, 4]], 1), mask4[:])
            nc.vector.reduce_sum(e2, sel[:], axis=mybir.AxisListType.X)
            nc.vector.tensor_scalar_mul(e2, e2, 1.0 / N_TOT)
            nc.vector.tensor_mul(msq, mean, mean)
            nc.vector.tensor_tensor(out=var, in0=e2, in1=msq,
                                    op=mybir.AluOpType.subtract)
            eps_ap = scalars[:, 13:14]
            nc.vector.memset(eps_ap, EPS)
            nc.scalar.activation(sd, var, mybir.ActivationFunctionType.Sqrt,
                                 bias=eps_ap)
            nc.vector.reciprocal(rstd, sd)
            nc.vector.tensor_scalar_mul(sc, rstd, NORM_SCALE)
            nc.vector.tensor_mul(bb0, mean, sc)
            nc.vector.tensor_scalar_mul(bb, bb0, -1.0)

        # ---------------- pass C: out = cm*x + sc*avp + bb ------------------
        if level >= 5:
            pc_t = tc.alloc_tile_pool(name="pc_t", bufs=2)
            pc_o = tc.alloc_tile_pool(name="pc_o", bufs=4)
            for idx, t in enumerate(PC_ORDER):
                y0 = t * R
                x_s = pcx_tiles.pop(t)
                if cm != 1.0:
                    xc = pc_t.tile([128, R, W], BF16, tag="xc")
                    nc.scalar.mul(xc[:], x_s[:], float(cm))
                    x_s = xc
                av_s = pca_tiles.pop(t)
                # t = sc*av + bb (Act and DVE alternate strips), out = t + x.
                t_s = pc_t.tile([128, R, W], BF16, tag="t_s")
                if idx % 2 == 0:
                    nc.scalar.activation(t_s[:], av_s[:],
                                         mybir.ActivationFunctionType.Identity,
                                         bias=bb, scale=sc)
                else:
                    nc.vector.tensor_scalar(out=t_s[:], in0=av_s[:],
                                            scalar1=sc, scalar2=bb,
                                            op0=mybir.AluOpType.mult,
                                            op1=mybir.AluOpType.add)
                o_s = pc_o.tile([128, R, W], BF16, tag="o_s")
                nc.vector.tensor_add(o_s[:], t_s[:], x_s[:])
                nc.sync.dma_start(out=out_d[:, y0:y0 + R, :], in_=o_s[:])
            pc_o.release(); pc_t.release()

        if level >= 3:
            cc_pool.release()
        if level >= 5:
            pc_x.release(); pc_a.release()
        small.release()
        consts.release()

    nc.compile()
    return nc


def _prep_wt(w, permute_out=False):
    """[Co,Ci,3,3] -> lhsT layout [Ci, 9, Co] (optionally out-chan permuted)."""
    import ml_dtypes
    wt = np.ascontiguousarray(w.transpose(1, 2, 3, 0).reshape(128, 9, 128))
    if permute_out:
        p = np.arange(128)
        co_of_p = 4 * (p % 32) + p // 32     # partition p holds channel co_of_p
        wt = np.ascontiguousarray(wt[:, :, co_of_p])
    return wt.astype(ml_dtypes.bfloat16)


def make_const_inputs(w1, w2, w3):
    import ml_dtypes
    p = np.arange(128)
    # gsum[p_src, C']: sum av partitions with p_src%32 == C'//4
    gsum = (p[:, None] % 32 == p[None, :] // 4).astype(np.float32)
    mask4 = (p[:, None] % 4 == np.arange(4)[None, :]).astype(np.float32)
    return {
        "ident": np.eye(128, dtype=np.float32).astype(ml_dtypes.bfloat16),
        "w1t": _prep_wt(np.asarray(w1, np.float32)),
        "w2t": _prep_wt(np.asarray(w2, np.float32), permute_out=True),
        "w3t": _prep_wt(np.asarray(w3, np.float32)),
        "gsum": gsum,
        "mask4": mask4,
    }


def pad_x(x_sample):
    import ml_dtypes
    return np.pad(x_sample, ((0, 0), (1, 1), (1, 1))).astype(ml_dtypes.bfloat16)


_CACHE = {}


def kernel(x, w1, w2, w3, conv_momentum):
    from concourse.bass_utils import run_bass_kernel_spmd

    x = np.asarray(x, np.float32)
    B, Ci, H, W = x.shape
    cm = float(np.asarray(conv_momentum))
    key = (H, W, B, cm)
    if key not in _CACHE:
        _CACHE[key] = build_nc(H, W, 24, B, cm)
    nc = _CACHE[key]
    consts = make_const_inputs(w1, w2, w3)
    in_maps = [dict(consts, x=pad_x(x[b])) for b in range(B)]
    res = run_bass_kernel_spmd(nc, in_maps, list(range(B)))
    out = np.stack(
        [np.asarray(res.results[b]["out"]).reshape(128, H, W) for b in range(B)],
        axis=0)
    return out.astype(np.float32)
